# revision 9
# baseline (speedup 1.0000x reference)
"""ChebGCN (K=2, 3 layers) Trainium2 Bass kernel.

Strategy (1D graph/data parallel, dst-sharded):
  - Host: convert edge list -> dense adjacency COUNT strips AdjT[src, dst_local]
    per core (fp8 e4m3: small integer counts are exact), pad N 10000->10240,
    shard dst rows 1280/core. Pure format conversion; all FP math on device.
  - Device (SPMD on 8 cores):
      dis = mask * 1/sqrt(max(deg,1)) computed on device from integer counts.
      Per layer, the Chebyshev term  L_hat x = -D A D x  is computed as a dense
      matmul on the tensor engine:  T^T = (dis .* X)^T @ AdjT  (fp16 x fp8),
      then scaled by -dis_dst on evacuation. Dense W0/W1 matmuls run
      feature-major; h stays feature-major, and is PE-transposed to node-major
      only to feed the inter-layer AllGather (8-core collective).
  - Layer 3 folds W13 before the gather (Y3 = (dis.*h2) @ W13), halving the
    final dense-adjacency matmul width.

kernel(**inputs) takes the FULL unsharded inputs and returns the FULL output.
"""

import os
import sys

sys.path.insert(0, "/opt/trn_rl_repo")

import numpy as np
import ml_dtypes

N = 10000
NP = 10240          # padded node count
NCORES = 8
MLOC = NP // NCORES  # 1280 dst rows per core
P = 128
KT = NP // P         # 80 source chunks of 128
TPC = MLOC // P      # 10 dst tiles per core
D_IN, D_HID, D_OUT = 128, 256, 128
CH = [(0, 512), (512, 512), (1024, 256)]  # dst column chunks (psum bank sized)

_CACHE = {}
LAST_RESULTS = None  # BassKernelResults of the most recent run (for profiling)


def _build_nc():
    from contextlib import ExitStack

    import concourse.bass as bass
    import concourse.tile as tile
    from concourse import bacc, mybir
    from concourse.masks import make_identity

    f32 = mybir.dt.float32
    f16 = mybir.dt.float16
    f8 = mybir.dt.float8e4
    AF = mybir.ActivationFunctionType
    MUL = mybir.AluOpType.mult

    nc = bacc.Bacc(trn_type="TRN2", num_devices=NCORES)

    adjT_d = nc.dram_tensor("adjT", [KT, P, MLOC], f8, kind="ExternalInput")
    x_nm_d = nc.dram_tensor("x_nm", [KT, P, D_IN], f32, kind="ExternalInput")
    xT_d = nc.dram_tensor("xT_loc", [P, MLOC], f32, kind="ExternalInput")
    degc_d = nc.dram_tensor("deg_cols", [P, KT], f32, kind="ExternalInput")
    degr_d = nc.dram_tensor("deg_row", [1, MLOC], f32, kind="ExternalInput")
    w01_d = nc.dram_tensor("w01", [P, D_HID], f32, kind="ExternalInput")
    w11_d = nc.dram_tensor("w11", [P, D_HID], f32, kind="ExternalInput")
    w02_d = nc.dram_tensor("w02", [2, P, D_HID], f32, kind="ExternalInput")
    w12_d = nc.dram_tensor("w12", [2, P, D_HID], f32, kind="ExternalInput")
    w03_d = nc.dram_tensor("w03", [2, P, D_OUT], f32, kind="ExternalInput")
    w13_d = nc.dram_tensor("w13", [2, P, D_OUT], f32, kind="ExternalInput")
    b1_d = nc.dram_tensor("b1r", [1, D_HID], f32, kind="ExternalInput")
    b2_d = nc.dram_tensor("b2r", [1, D_HID], f32, kind="ExternalInput")
    b3_d = nc.dram_tensor("b3r", [1, D_OUT], f32, kind="ExternalInput")
    out_d = nc.dram_tensor("outT", [P, MLOC], f32, kind="ExternalOutput")

    with tile.TileContext(nc) as tc, ExitStack() as ctx:
        const = ctx.enter_context(tc.tile_pool(name="const", bufs=1))
        stage = ctx.enter_context(tc.tile_pool(name="stage", bufs=1))
        io = ctx.enter_context(tc.tile_pool(name="io", bufs=3))
        adjp = ctx.enter_context(tc.tile_pool(name="adjp", bufs=4))
        feat = ctx.enter_context(tc.tile_pool(name="feat", bufs=1))
        pbig = ctx.enter_context(tc.tile_pool(name="pbig", bufs=4, space="PSUM"))
        pout = ctx.enter_context(tc.tile_pool(name="pout", bufs=2, space="PSUM"))
        ptr = ctx.enter_context(tc.tile_pool(name="ptr", bufs=2, space="PSUM"))
        dram = ctx.enter_context(tc.tile_pool(name="dram", bufs=1, space="DRAM"))

        adjT = adjT_d[:]

        # ---------- constants ----------
        id16 = const.tile([P, P], f16)
        make_identity(nc, id16)
        id32 = const.tile([P, P], f32)
        make_identity(nc, id32)
        ones1f = const.tile([1, P], f32)
        nc.gpsimd.memset(ones1f[:], 1.0)
        onesrow = const.tile([1, MLOC], f16)
        nc.gpsimd.memset(onesrow[:], 1.0)

        # ---------- weights -> fp16 ----------
        def load_cast(name, dtensor, shape):
            wf = stage.tile(shape, f32, name=f"{name}_f")
            nc.sync.dma_start(wf[:], dtensor[:])
            wh = const.tile(shape, f16, name=name)
            nc.vector.tensor_copy(wh[:], wf[:])
            return wh

        w01h = load_cast("w01h", w01_d, [P, D_HID])
        w11h = load_cast("w11h", w11_d, [P, D_HID])
        w02h = load_cast("w02h", w02_d[:].rearrange("b p w -> p b w"), [P, 2, D_HID])
        w12h = load_cast("w12h", w12_d[:].rearrange("b p w -> p b w"), [P, 2, D_HID])
        w03h = load_cast("w03h", w03_d[:].rearrange("b p w -> p b w"), [P, 2, D_OUT])
        w13h = load_cast("w13h", w13_d[:].rearrange("b p w -> p b w"), [P, 2, D_OUT])
        b1h = load_cast("b1h", b1_d, [1, D_HID])
        b2h = load_cast("b2h", b2_d, [1, D_HID])
        b3h = load_cast("b3h", b3_d, [1, D_OUT])

        # ---------- degree -> dis on device ----------
        def make_dis(name, dtensor, shape):
            # dis = sqrt(min(deg,1) * 1/max(deg,1)); all-DVE chain, one ACT sqrt
            dg = stage.tile(shape, f32, name=f"{name}_dg")
            nc.sync.dma_start(dg[:], dtensor[:])
            tmp = stage.tile(shape, f32, name=f"{name}_tmp")
            nc.vector.tensor_scalar_max(tmp[:], dg[:], 1.0)
            nc.vector.reciprocal(tmp[:], tmp[:])
            msk = stage.tile(shape, f32, name=f"{name}_msk")
            nc.vector.tensor_scalar_min(msk[:], dg[:], 1.0)
            nc.vector.tensor_tensor(tmp[:], tmp[:], msk[:], MUL)
            dis = const.tile(shape, f32, name=name)
            nc.scalar.activation(dis[:], tmp[:], AF.Sqrt)
            return dis

        dis_cols = make_dis("dis_cols", degc_d, [P, KT])      # dis over all 10240 src
        dis_row = make_dis("dis_row", degr_d, [1, MLOC])      # dis over local 1280 dst

        # ndis_bc[p, j] = -dis_row[j]  (broadcast via K=1 matmul)
        ndis_row = const.tile([1, MLOC], f32)
        nc.vector.tensor_scalar_mul(ndis_row[:], dis_row[:], -1.0)
        ndis_bc = const.tile([P, MLOC], f32)
        pdis_bc = const.tile([P, MLOC], f16)
        for c0, cw in CH:
            pb = pout.tile([P, 512], f32, name="pb_bc", tag="po")
            nc.tensor.matmul(pb[:, :cw], ones1f[:], ndis_row[:, c0 : c0 + cw])
            nc.vector.tensor_copy(ndis_bc[:, c0 : c0 + cw], pb[:, :cw])
            nc.vector.tensor_scalar_mul(pdis_bc[:, c0 : c0 + cw], pb[:, :cw], -1.0)

        # ---------- x: scaled node-major (Xs1) + local feature-major ----------
        xs1 = feat.tile([P, KT, D_IN], f16, tag="nmfeat")
        x_nm = x_nm_d[:]
        for k in range(KT):
            xkf = io.tile([P, D_IN], f32, name="xkf")
            nc.sync.dma_start(xkf[:], x_nm[k])
            nc.scalar.activation(
                xs1[:, k, :], xkf[:], AF.Copy, scale=dis_cols[:, k : k + 1]
            )
        xT16 = const.tile([P, MLOC], f16)
        xTf = stage.tile([P, MLOC], f32, name="xTf")
        nc.sync.dma_start(xTf[:], xT_d[:])
        nc.vector.tensor_copy(xT16[:], xTf[:])

        # ---------- helper: dense-adjacency matmul over one fp16 feature block ----
        def big_pass(feat_tiles_block, ts_out_slices, out_dtype_f32=False):
            """feat_tiles_block: fn(k) -> AP [P, 128] fp16 (lhsT for chunk k).
            ts_out_slices: fn(c0, cw) -> AP destination [P, cw] for scaled T^T."""
            accs = [pbig.tile([P, 512], f32, name="acc") for _ in CH]
            for k in range(KT):
                at = adjp.tile([P, MLOC], f8, name="at")
                nc.sync.dma_start(at[:], adjT[k])
                for i, (c0, cw) in enumerate(CH):
                    nc.tensor.matmul(
                        accs[i][:, :cw],
                        feat_tiles_block(k),
                        at[:, c0 : c0 + cw],
                        start=(k == 0),
                        stop=(k == KT - 1),
                    )
            for i, (c0, cw) in enumerate(CH):
                nc.vector.tensor_tensor(
                    ts_out_slices(c0, cw), accs[i][:, :cw], ndis_bc[:, c0 : c0 + cw], MUL
                )

        # ================= Layer 1 =================
        t1s = feat.tile([P, MLOC], f16)
        big_pass(lambda k: xs1[:, k, :], lambda c0, cw: t1s[:, c0 : c0 + cw])

        h1T = feat.tile([P, 2, MLOC], f16)
        for bo in range(2):
            bs = slice(bo * P, (bo + 1) * P)
            for c0, cw in CH:
                po = pout.tile([P, 512], f32, name="po")
                cs = slice(c0, c0 + cw)
                nc.tensor.matmul(po[:, :cw], w01h[:, bs], xT16[:, cs], start=True, stop=False)
                nc.tensor.matmul(po[:, :cw], w11h[:, bs], t1s[:, cs], start=False, stop=False)
                nc.tensor.matmul(po[:, :cw], b1h[:, bs], onesrow[:, cs], start=False, stop=True)
                nc.scalar.activation(h1T[:, bo, cs], po[:, :cw], AF.Relu)

        # h1s (scaled) -> node-major -> bounce -> AllGather
        h1sT = feat.tile([P, 2, MLOC], f16)
        for b in range(2):
            nc.vector.tensor_tensor(h1sT[:, b, :], h1T[:, b, :], pdis_bc[:], MUL)
        h1s_bounce = dram.tile([MLOC, D_HID], f16, name="h1s_bounce")
        h1s_bounce_v = h1s_bounce[:].rearrange("(t p) d -> p t d", p=P)
        for t in range(TPC):
            hst = stage.tile([P, D_HID], f16, name="hst")
            for b in range(2):
                ptt = ptr.tile([P, P], f16, name="ptt")
                nc.tensor.transpose(ptt[:], h1sT[:, b, t * P : (t + 1) * P], id16[:])
                nc.vector.tensor_copy(hst[:, b * P : (b + 1) * P], ptt[:])
            nc.sync.dma_start(h1s_bounce_v[:, t, :], hst[:])
        h1s_full = dram.tile([NP, D_HID], f16, name="h1s_full", addr_space="Shared")
        nc.gpsimd.collective_compute(
            "AllGather",
            mybir.AluOpType.bypass,
            replica_groups=[list(range(NCORES))],
            ins=[h1s_bounce[:]],
            outs=[h1s_full[:]],
        )

        # ================= Layer 2 =================
        y2f = feat.tile([P, KT, D_HID], f16)
        nc.sync.dma_start(y2f[:], h1s_full[:].rearrange("(k p) d -> p k d", p=P))
        t2s = feat.tile([P, 2, MLOC], f16)
        for b in range(2):
            bs = slice(b * P, (b + 1) * P)
            big_pass(
                lambda k, bs=bs: y2f[:, k, bs],
                lambda c0, cw, b=b: t2s[:, b, c0 : c0 + cw],
            )

        h2T = feat.tile([P, 2, MLOC], f16)
        for bo in range(2):
            bs = slice(bo * P, (bo + 1) * P)
            for c0, cw in CH:
                po = pout.tile([P, 512], f32, name="po")
                cs = slice(c0, c0 + cw)
                nc.tensor.matmul(po[:, :cw], w02h[:, 0, bs], h1T[:, 0, cs], start=True, stop=False)
                nc.tensor.matmul(po[:, :cw], w02h[:, 1, bs], h1T[:, 1, cs], start=False, stop=False)
                nc.tensor.matmul(po[:, :cw], w12h[:, 0, bs], t2s[:, 0, cs], start=False, stop=False)
                nc.tensor.matmul(po[:, :cw], w12h[:, 1, bs], t2s[:, 1, cs], start=False, stop=False)
                nc.tensor.matmul(po[:, :cw], b2h[:, bs], onesrow[:, cs], start=False, stop=True)
                nc.scalar.activation(h2T[:, bo, cs], po[:, :cw], AF.Relu)

        # Y3 = (dis .* h2) @ W13, feature-major then node-major -> gather
        h2sT = feat.tile([P, 2, MLOC], f16)
        for b in range(2):
            nc.vector.tensor_tensor(h2sT[:, b, :], h2T[:, b, :], pdis_bc[:], MUL)
        y3T = feat.tile([P, MLOC], f16)
        for c0, cw in CH:
            py = pout.tile([P, 512], f32, name="po")
            cs = slice(c0, c0 + cw)
            nc.tensor.matmul(py[:, :cw], w13h[:, 0, :], h2sT[:, 0, cs], start=True, stop=False)
            nc.tensor.matmul(py[:, :cw], w13h[:, 1, :], h2sT[:, 1, cs], start=False, stop=True)
            nc.vector.tensor_copy(y3T[:, cs], py[:, :cw])
        y3_bounce = dram.tile([MLOC, D_OUT], f16, name="y3_bounce")
        y3_bounce_v = y3_bounce[:].rearrange("(t p) d -> p t d", p=P)
        for t in range(TPC):
            y3t = stage.tile([P, D_OUT], f16, name="y3t")
            ptt = ptr.tile([P, P], f16, name="ptt")
            nc.tensor.transpose(ptt[:], y3T[:, t * P : (t + 1) * P], id16[:])
            nc.vector.tensor_copy(y3t[:], ptt[:])
            nc.sync.dma_start(y3_bounce_v[:, t, :], y3t[:])
        y3_full = dram.tile([NP, D_OUT], f16, name="y3_full", addr_space="Shared")
        nc.gpsimd.collective_compute(
            "AllGather",
            mybir.AluOpType.bypass,
            replica_groups=[list(range(NCORES))],
            ins=[y3_bounce[:]],
            outs=[y3_full[:]],
        )

        # ================= Layer 3 =================
        y3f = feat.tile([P, KT, D_OUT], f16)
        nc.sync.dma_start(y3f[:], y3_full[:].rearrange("(k p) d -> p k d", p=P))
        t3s = feat.tile([P, MLOC], f32)
        big_pass(lambda k: y3f[:, k, :], lambda c0, cw: t3s[:, c0 : c0 + cw])

        outT = feat.tile([P, MLOC], f32)
        for c0, cw in CH:
            po = pout.tile([P, 512], f32, name="po")
            cs = slice(c0, c0 + cw)
            nc.tensor.matmul(po[:, :cw], w03h[:, 0, :], h2T[:, 0, cs], start=True, stop=False)
            nc.tensor.matmul(po[:, :cw], w03h[:, 1, :], h2T[:, 1, cs], start=False, stop=False)
            nc.tensor.matmul(po[:, :cw], b3h[:], onesrow[:, cs], start=False, stop=False)
            # += T3s (identity-matmul add of the scaled Chebyshev term)
            nc.tensor.matmul(po[:, :cw], id32[:], t3s[:, cs], start=False, stop=True)
            nc.vector.tensor_copy(outT[:, cs], po[:, :cw])
        nc.sync.dma_start(out_d[:], outT[:])

    nc.compile()
    return nc


def _prep_inputs(x, edge_index, W01, W11, b1, W02, W12, b2, W03, W13, b3):
    f8 = ml_dtypes.float8_e4m3
    x = np.asarray(x, np.float32)
    ei = np.asarray(edge_index)
    src = ei[0].astype(np.int64)
    dst = ei[1].astype(np.int64)

    deg = np.bincount(src, minlength=NP).astype(np.float32)  # out-degree counts
    x_pad = np.zeros((NP, D_IN), np.float32)
    x_pad[:N] = x

    x_nm = np.ascontiguousarray(x_pad.reshape(KT, P, D_IN))
    deg_cols = np.ascontiguousarray(deg.reshape(KT, P).T)

    common = {
        "x_nm": x_nm,
        "deg_cols": deg_cols,
        "w01": np.ascontiguousarray(np.asarray(W01, np.float32)),
        "w11": np.ascontiguousarray(np.asarray(W11, np.float32)),
        "w02": np.ascontiguousarray(np.asarray(W02, np.float32).reshape(2, P, D_HID)),
        "w12": np.ascontiguousarray(np.asarray(W12, np.float32).reshape(2, P, D_HID)),
        "w03": np.ascontiguousarray(np.asarray(W03, np.float32).reshape(2, P, D_OUT)),
        "w13": np.ascontiguousarray(np.asarray(W13, np.float32).reshape(2, P, D_OUT)),
        "b1r": np.asarray(b1, np.float32).reshape(1, D_HID),
        "b2r": np.asarray(b2, np.float32).reshape(1, D_HID),
        "b3r": np.asarray(b3, np.float32).reshape(1, D_OUT),
    }

    in_maps = []
    for c in range(NCORES):
        lo, hi = c * MLOC, (c + 1) * MLOC
        sel = (dst >= lo) & (dst < hi)
        idx = src[sel] * MLOC + (dst[sel] - lo)
        cnt = np.bincount(idx, minlength=NP * MLOC).astype(np.float32)
        adjT = cnt.reshape(KT, P, MLOC).astype(f8)
        m = dict(common)
        m["adjT"] = adjT
        m["xT_loc"] = np.ascontiguousarray(x_pad[lo:hi].T)
        m["deg_row"] = np.ascontiguousarray(deg[lo:hi].reshape(1, MLOC))
        in_maps.append(m)
    return in_maps


def kernel(x, edge_index, edge_type, W01, W11, b1, W02, W12, b2, W03, W13, b3):
    global LAST_RESULTS
    from concourse.bass_utils import run_bass_kernel_spmd

    if "nc" not in _CACHE:
        _CACHE["nc"] = _build_nc()
    nc = _CACHE["nc"]

    in_maps = _prep_inputs(x, edge_index, W01, W11, b1, W02, W12, b2, W03, W13, b3)
    res = run_bass_kernel_spmd(
        nc,
        in_maps,
        list(range(NCORES)),
        trace=bool(os.environ.get("BASS_TRACE")),
    )
    LAST_RESULTS = res
    shards = [res.results[c]["outT"].astype(np.float32).T for c in range(NCORES)]
    out = np.concatenate(shards, axis=0)[:N]
    return np.ascontiguousarray(out)


if __name__ == "__main__":
    # smoke build
    _build_nc()
    print("build ok")


# revision 12
# speedup vs baseline: 1.0862x; 1.0862x over previous
"""ChebGCN (K=2, 3 layers) Trainium2 Bass kernel.

Strategy (1D graph/data parallel, dst-sharded):
  - Host: convert edge list -> dense adjacency COUNT strips AdjT[src, dst_local]
    per core (fp8 e4m3: small integer counts are exact), pad N 10000->10240,
    shard dst rows 1280/core. Pure format conversion; all FP math on device.
  - Device (SPMD on 8 cores):
      dis = sqrt(min(deg,1)/max(deg,1)) computed on device from integer counts.
      Per layer, the Chebyshev term  L_hat x = -D A D x  is computed as a dense
      matmul on the tensor engine:  T^T = (dis .* X)^T @ AdjT  (fp16 x fp8),
      scaled by -dis_dst on PSUM evacuation. Dense W0/W1 matmuls run
      feature-major; layer outputs are PE-transposed to node-major only to
      feed the inter-layer AllGather.
  - The AllGather at each layer boundary is split into 5 pieces (2 dst tiles
    per core each) that pipeline against the next layer's adjacency matmul.
    Source chunks are consumed in a host-side permutation (sigma) so each
    gathered piece maps to a contiguous run of contraction chunks.
  - Layer 3 folds W13 before the gather (Y3 = (dis.*h2) @ W13), halving the
    final dense-adjacency matmul width.

kernel(**inputs) takes the FULL unsharded inputs and returns the FULL output.
"""

import os
import sys

sys.path.insert(0, "/opt/trn_rl_repo")

import numpy as np
import ml_dtypes

N = 10000
NP = 10240           # padded node count
NCORES = 8
MLOC = NP // NCORES  # 1280 dst rows per core
P = 128
KT = NP // P         # 80 source chunks of 128
TPC = MLOC // P      # 10 dst tiles per core
D_IN, D_HID, D_OUT = 128, 256, 128
CH = [(0, 512), (512, 512), (1024, 256)]  # dst column chunks (psum bank sized)
NPIECE = 5           # gather pieces per boundary
GPP = KT // NPIECE   # global chunks per piece (16)
PIECE_ROWS = 2 * P   # local rows per piece (2 dst tiles)

# position j = p*16 + c*2 + u  <->  global chunk c*10 + 2p + u
SIGMA = [c * TPC + 2 * p + u for p in range(NPIECE) for c in range(NCORES) for u in range(2)]

_CACHE = {}
LAST_RESULTS = None  # BassKernelResults of the most recent run (for profiling)


def _build_nc():
    from contextlib import ExitStack

    import concourse.bass as bass
    import concourse.tile as tile
    from concourse import bacc, mybir
    from concourse.masks import make_identity

    f32 = mybir.dt.float32
    f16 = mybir.dt.float16
    f8 = mybir.dt.float8e4
    AF = mybir.ActivationFunctionType
    MUL = mybir.AluOpType.mult

    nc = bacc.Bacc(trn_type="TRN2", num_devices=NCORES)

    adjT_d = nc.dram_tensor("adjT", [KT, P, MLOC], f8, kind="ExternalInput")
    x_nm_d = nc.dram_tensor("x_nm", [KT, P, D_IN], f32, kind="ExternalInput")
    xT_d = nc.dram_tensor("xT_loc", [P, MLOC], f32, kind="ExternalInput")
    degc_d = nc.dram_tensor("deg_cols", [P, KT], f32, kind="ExternalInput")
    degr_d = nc.dram_tensor("deg_row", [1, MLOC], f32, kind="ExternalInput")
    w01_d = nc.dram_tensor("w01", [P, D_HID], f32, kind="ExternalInput")
    w11_d = nc.dram_tensor("w11", [P, D_HID], f32, kind="ExternalInput")
    w02_d = nc.dram_tensor("w02", [2, P, D_HID], f32, kind="ExternalInput")
    w12_d = nc.dram_tensor("w12", [2, P, D_HID], f32, kind="ExternalInput")
    w03_d = nc.dram_tensor("w03", [2, P, D_OUT], f32, kind="ExternalInput")
    w13_d = nc.dram_tensor("w13", [2, P, D_OUT], f32, kind="ExternalInput")
    b1_d = nc.dram_tensor("b1r", [1, D_HID], f32, kind="ExternalInput")
    b2_d = nc.dram_tensor("b2r", [1, D_HID], f32, kind="ExternalInput")
    b3_d = nc.dram_tensor("b3r", [1, D_OUT], f32, kind="ExternalInput")
    out_d = nc.dram_tensor("outT", [P, MLOC], f32, kind="ExternalOutput")

    with tile.TileContext(nc) as tc, ExitStack() as ctx:
        const = ctx.enter_context(tc.tile_pool(name="const", bufs=1))
        stage = ctx.enter_context(tc.tile_pool(name="stage", bufs=1))
        io = ctx.enter_context(tc.tile_pool(name="io", bufs=4))
        adjp = ctx.enter_context(tc.tile_pool(name="adjp", bufs=4))
        feat = ctx.enter_context(tc.tile_pool(name="feat", bufs=1))
        pbig = ctx.enter_context(tc.tile_pool(name="pbig", bufs=3, space="PSUM"))
        pout = ctx.enter_context(tc.tile_pool(name="pout", bufs=3, space="PSUM"))
        ptr = ctx.enter_context(tc.tile_pool(name="ptr", bufs=2, space="PSUM"))
        dram = ctx.enter_context(tc.tile_pool(name="dram", bufs=1, space="DRAM"))

        adjT = adjT_d[:]
        x_nm = x_nm_d[:]

        # ---------- degree -> dis on device (emitted first: feeds L1) -------
        def make_dis(name, dtensor, shape):
            # dis = sqrt(min(deg,1) * 1/max(deg,1)); all-DVE chain, one ACT sqrt
            dg = stage.tile(shape, f32, name=f"{name}_dg")
            nc.sync.dma_start(dg[:], dtensor[:])
            tmp = stage.tile(shape, f32, name=f"{name}_tmp")
            nc.vector.tensor_scalar_max(tmp[:], dg[:], 1.0)
            nc.vector.reciprocal(tmp[:], tmp[:])
            msk = stage.tile(shape, f32, name=f"{name}_msk")
            nc.vector.tensor_scalar_min(msk[:], dg[:], 1.0)
            nc.vector.tensor_tensor(tmp[:], tmp[:], msk[:], MUL)
            dis = const.tile(shape, f32, name=name)
            nc.scalar.activation(dis[:], tmp[:], AF.Sqrt)
            return dis

        dis_cols = make_dis("dis_cols", degc_d, [P, KT])  # dis over all src (sigma order)
        dis_row = make_dis("dis_row", degr_d, [1, MLOC])  # dis over local dst

        # ---------- x: scaled node-major pieces (DVE) ----------
        xs1p = [feat.tile([P, GPP, D_IN], f16, name=f"xs1_{p}") for p in range(NPIECE)]
        for j in range(KT):
            xkf = io.tile([P, D_IN], f32, name="xkf")
            nc.sync.dma_start(xkf[:], x_nm[j])
            nc.vector.tensor_scalar(
                out=xs1p[j // GPP][:, j % GPP, :],
                in0=xkf[:],
                scalar1=dis_cols[:, j : j + 1],
                scalar2=None,
                op0=MUL,
            )

        # ---------- helper: dense-adjacency matmul over an fp16 block -------
        def big_pass(feat_block, ts_out, tag):
            """feat_block(j) -> AP [P,128] fp16 lhsT; ts_out(c0,cw) -> dest AP."""
            accs = [pbig.tile([P, 512], f32, name=f"acc{i}_{tag}", tag="acc") for i in range(3)]
            for j in range(KT):
                at = adjp.tile([P, MLOC], f8, name="at")
                nc.sync.dma_start(at[:], adjT[j])
                for i, (c0, cw) in enumerate(CH):
                    nc.tensor.matmul(
                        accs[i][:, :cw],
                        feat_block(j),
                        at[:, c0 : c0 + cw],
                        start=(j == 0),
                        stop=(j == KT - 1),
                    )
            for i, (c0, cw) in enumerate(CH):
                nc.vector.tensor_tensor(
                    ts_out(c0, cw), accs[i][:, :cw], ndis_bc[:, c0 : c0 + cw], MUL
                )

        # broadcast rows (must be WRITTEN before the L1 evacuation reads them —
        # Tile dependencies follow emission order)
        ones1f = const.tile([1, P], f32)
        nc.gpsimd.memset(ones1f[:], 1.0)
        ndis_row = const.tile([1, MLOC], f32)
        nc.vector.tensor_scalar_mul(ndis_row[:], dis_row[:], -1.0)
        ndis_bc = const.tile([P, MLOC], f32)
        pdis_bc = const.tile([P, MLOC], f16)
        for c0, cw in CH:
            pb = pout.tile([P, 512], f32, name="pb_bc", tag="po")
            nc.tensor.matmul(pb[:, :cw], ones1f[:], ndis_row[:, c0 : c0 + cw])
            nc.vector.tensor_copy(ndis_bc[:, c0 : c0 + cw], pb[:, :cw])
            nc.vector.tensor_scalar_mul(pdis_bc[:, c0 : c0 + cw], pb[:, :cw], -1.0)

        # ================= Layer 1 big matmul (emitted early) ===============
        t1s = feat.tile([P, MLOC], f16)
        big_pass(lambda j: xs1p[j // GPP][:, j % GPP, :], lambda c0, cw: t1s[:, c0 : c0 + cw], "l1")

        # ---------- constants / weights (scheduler fills these in) ----------
        id16 = const.tile([P, P], f16)
        make_identity(nc, id16)
        id32 = const.tile([P, P], f32)
        make_identity(nc, id32)
        onesrow = const.tile([1, MLOC], f16)
        nc.gpsimd.memset(onesrow[:], 1.0)

        def load_cast(name, dtensor, shape):
            wf = stage.tile(shape, f32, name=f"{name}_f")
            nc.sync.dma_start(wf[:], dtensor[:])
            wh = const.tile(shape, f16, name=name)
            nc.vector.tensor_copy(wh[:], wf[:])
            return wh

        w01h = load_cast("w01h", w01_d, [P, D_HID])
        w11h = load_cast("w11h", w11_d, [P, D_HID])
        w02h = load_cast("w02h", w02_d[:].rearrange("b p w -> p b w"), [P, 2, D_HID])
        w12h = load_cast("w12h", w12_d[:].rearrange("b p w -> p b w"), [P, 2, D_HID])
        w03h = load_cast("w03h", w03_d[:].rearrange("b p w -> p b w"), [P, 2, D_OUT])
        w13h = load_cast("w13h", w13_d[:].rearrange("b p w -> p b w"), [P, 2, D_OUT])
        b1h = load_cast("b1h", b1_d, [1, D_HID])
        b2h = load_cast("b2h", b2_d, [1, D_HID])
        b3h = load_cast("b3h", b3_d, [1, D_OUT])

        xT16 = const.tile([P, MLOC], f16)
        xTf = stage.tile([P, MLOC], f32, name="xTf")
        nc.sync.dma_start(xTf[:], xT_d[:])
        nc.vector.tensor_copy(xT16[:], xTf[:])

        # helper: transpose 128-col blocks of a feature-major tensor into a
        # node-major stage tile and DMA to a bounce-row block
        def emit_piece(srcT, blocks, width, bounce_v, p, gather_out, bounce, gathered):
            """srcT: fn(b, t) -> AP [P, P] fp16 (feature block b, dst tile t).
            blocks: number of 128-row feature blocks (width // 128)."""
            for t in (2 * p, 2 * p + 1):
                hst = io.tile([P, width], f16, name="hst", tag="hst")
                for b in range(blocks):
                    ptt = ptr.tile([P, P], f16, name="ptt")
                    nc.tensor.transpose(ptt[:], srcT(b, t), id16[:])
                    nc.vector.tensor_copy(hst[:, b * P : (b + 1) * P], ptt[:])
                nc.sync.dma_start(bounce_v[:, t, :], hst[:])
            nc.gpsimd.collective_compute(
                "AllGather",
                mybir.AluOpType.bypass,
                replica_groups=[list(range(NCORES))],
                ins=[bounce[PIECE_ROWS * p : PIECE_ROWS * (p + 1), :]],
                outs=[gathered[:]],
            )
            gp = gather_out
            nc.sync.dma_start(gp[:], gathered[:].rearrange("(g q) d -> q g d", q=P))

        # ================= Layer 1 dense part =================
        h1T = feat.tile([P, 2, MLOC], f16)
        h1sT = feat.tile([P, 2, MLOC], f16)
        h1s_bounce = dram.tile([MLOC, D_HID], f16, name="h1s_bounce")
        h1s_bounce_v = h1s_bounce[:].rearrange("(t q) d -> q t d", q=P)
        y2fp = [feat.tile([P, GPP, D_HID], f16, name=f"y2f_{p}") for p in range(NPIECE)]
        h1s_gath = [
            dram.tile([GPP * P, D_HID], f16, name=f"h1s_gath{p}", addr_space="Shared")
            for p in range(NPIECE)
        ]

        piece_of_chunk = {0: (0, 1), 1: (2, 3), 2: (4,)}
        for ci, (c0, cw) in enumerate(CH):
            cs = slice(c0, c0 + cw)
            for bo in range(2):
                bs = slice(bo * P, (bo + 1) * P)
                po = pout.tile([P, 512], f32, name="po")
                nc.tensor.matmul(po[:, :cw], w01h[:, bs], xT16[:, cs], start=True, stop=False)
                nc.tensor.matmul(po[:, :cw], w11h[:, bs], t1s[:, cs], start=False, stop=False)
                nc.tensor.matmul(po[:, :cw], b1h[:, bs], onesrow[:, cs], start=False, stop=True)
                nc.vector.tensor_relu(h1T[:, bo, cs], po[:, :cw])
            for b in range(2):
                nc.vector.tensor_tensor(h1sT[:, b, cs], h1T[:, b, cs], pdis_bc[:, cs], MUL)
            for p in piece_of_chunk[ci]:
                emit_piece(
                    lambda b, t: h1sT[:, b, t * P : (t + 1) * P],
                    2,
                    D_HID,
                    h1s_bounce_v,
                    p,
                    y2fp[p][:],
                    h1s_bounce,
                    h1s_gath[p],
                )

        # ================= Layer 2 =================
        t2s = feat.tile([P, 2, MLOC], f16)
        for b in range(2):
            bs = slice(b * P, (b + 1) * P)
            big_pass(
                lambda j, bs=bs: y2fp[j // GPP][:, j % GPP, bs],
                lambda c0, cw, b=b: t2s[:, b, c0 : c0 + cw],
                f"l2_{b}",
            )

        h2T = feat.tile([P, 2, MLOC], f16)
        h2sT = feat.tile([P, 2, MLOC], f16)
        y3T = feat.tile([P, MLOC], f16)
        y3_bounce = dram.tile([MLOC, D_OUT], f16, name="y3_bounce")
        y3_bounce_v = y3_bounce[:].rearrange("(t q) d -> q t d", q=P)
        y3fp = [feat.tile([P, GPP, D_OUT], f16, name=f"y3f_{p}") for p in range(NPIECE)]
        y3_gath = [
            dram.tile([GPP * P, D_OUT], f16, name=f"y3_gath{p}", addr_space="Shared")
            for p in range(NPIECE)
        ]

        for ci, (c0, cw) in enumerate(CH):
            cs = slice(c0, c0 + cw)
            for bo in range(2):
                bs = slice(bo * P, (bo + 1) * P)
                po = pout.tile([P, 512], f32, name="po")
                nc.tensor.matmul(po[:, :cw], w02h[:, 0, bs], h1T[:, 0, cs], start=True, stop=False)
                nc.tensor.matmul(po[:, :cw], w02h[:, 1, bs], h1T[:, 1, cs], start=False, stop=False)
                nc.tensor.matmul(po[:, :cw], w12h[:, 0, bs], t2s[:, 0, cs], start=False, stop=False)
                nc.tensor.matmul(po[:, :cw], w12h[:, 1, bs], t2s[:, 1, cs], start=False, stop=False)
                nc.tensor.matmul(po[:, :cw], b2h[:, bs], onesrow[:, cs], start=False, stop=True)
                nc.vector.tensor_relu(h2T[:, bo, cs], po[:, :cw])
            for b in range(2):
                nc.vector.tensor_tensor(h2sT[:, b, cs], h2T[:, b, cs], pdis_bc[:, cs], MUL)
            # Y3 = (dis .* h2) @ W13, feature-major
            py = pout.tile([P, 512], f32, name="po")
            nc.tensor.matmul(py[:, :cw], w13h[:, 0, :], h2sT[:, 0, cs], start=True, stop=False)
            nc.tensor.matmul(py[:, :cw], w13h[:, 1, :], h2sT[:, 1, cs], start=False, stop=True)
            nc.vector.tensor_copy(y3T[:, cs], py[:, :cw])
            for p in piece_of_chunk[ci]:
                emit_piece(
                    lambda b, t: y3T[:, t * P : (t + 1) * P],
                    1,
                    D_OUT,
                    y3_bounce_v,
                    p,
                    y3fp[p][:],
                    y3_bounce,
                    y3_gath[p],
                )

        # ================= Layer 3 =================
        t3s = feat.tile([P, MLOC], f32)
        big_pass(lambda j: y3fp[j // GPP][:, j % GPP, :], lambda c0, cw: t3s[:, c0 : c0 + cw], "l3")

        outT = feat.tile([P, MLOC], f32)
        for c0, cw in CH:
            cs = slice(c0, c0 + cw)
            po = pout.tile([P, 512], f32, name="po")
            nc.tensor.matmul(po[:, :cw], w03h[:, 0, :], h2T[:, 0, cs], start=True, stop=False)
            nc.tensor.matmul(po[:, :cw], w03h[:, 1, :], h2T[:, 1, cs], start=False, stop=False)
            nc.tensor.matmul(po[:, :cw], b3h[:], onesrow[:, cs], start=False, stop=False)
            # += T3s (identity-matmul add of the scaled Chebyshev term)
            nc.tensor.matmul(po[:, :cw], id32[:], t3s[:, cs], start=False, stop=True)
            nc.vector.tensor_copy(outT[:, cs], po[:, :cw])
        nc.sync.dma_start(out_d[:], outT[:])

    nc.compile()
    return nc


def _prep_inputs(x, edge_index, W01, W11, b1, W02, W12, b2, W03, W13, b3):
    f8 = ml_dtypes.float8_e4m3
    x = np.asarray(x, np.float32)
    ei = np.asarray(edge_index)
    src = ei[0].astype(np.int64)
    dst = ei[1].astype(np.int64)

    deg = np.bincount(src, minlength=NP).astype(np.float32)  # out-degree counts
    x_pad = np.zeros((NP, D_IN), np.float32)
    x_pad[:N] = x

    sig = np.asarray(SIGMA)
    x_nm = np.ascontiguousarray(x_pad.reshape(KT, P, D_IN)[sig])
    deg_cols = np.ascontiguousarray(deg.reshape(KT, P)[sig].T)

    common = {
        "x_nm": x_nm,
        "deg_cols": deg_cols,
        "w01": np.ascontiguousarray(np.asarray(W01, np.float32)),
        "w11": np.ascontiguousarray(np.asarray(W11, np.float32)),
        "w02": np.ascontiguousarray(np.asarray(W02, np.float32).reshape(2, P, D_HID)),
        "w12": np.ascontiguousarray(np.asarray(W12, np.float32).reshape(2, P, D_HID)),
        "w03": np.ascontiguousarray(np.asarray(W03, np.float32).reshape(2, P, D_OUT)),
        "w13": np.ascontiguousarray(np.asarray(W13, np.float32).reshape(2, P, D_OUT)),
        "b1r": np.asarray(b1, np.float32).reshape(1, D_HID),
        "b2r": np.asarray(b2, np.float32).reshape(1, D_HID),
        "b3r": np.asarray(b3, np.float32).reshape(1, D_OUT),
    }

    in_maps = []
    for c in range(NCORES):
        lo, hi = c * MLOC, (c + 1) * MLOC
        sel = (dst >= lo) & (dst < hi)
        idx = src[sel] * MLOC + (dst[sel] - lo)
        cnt = np.bincount(idx, minlength=NP * MLOC).astype(np.float32)
        adjT = np.ascontiguousarray(cnt.reshape(KT, P, MLOC)[sig]).astype(f8)
        m = dict(common)
        m["adjT"] = adjT
        m["xT_loc"] = np.ascontiguousarray(x_pad[lo:hi].T)
        m["deg_row"] = np.ascontiguousarray(deg[lo:hi].reshape(1, MLOC))
        in_maps.append(m)
    return in_maps


def kernel(x, edge_index, edge_type, W01, W11, b1, W02, W12, b2, W03, W13, b3):
    global LAST_RESULTS
    from concourse.bass_utils import run_bass_kernel_spmd

    if "nc" not in _CACHE:
        _CACHE["nc"] = _build_nc()
    nc = _CACHE["nc"]

    in_maps = _prep_inputs(x, edge_index, W01, W11, b1, W02, W12, b2, W03, W13, b3)
    res = run_bass_kernel_spmd(
        nc,
        in_maps,
        list(range(NCORES)),
        trace=bool(os.environ.get("BASS_TRACE")),
    )
    LAST_RESULTS = res
    shards = [res.results[c]["outT"].astype(np.float32).T for c in range(NCORES)]
    out = np.concatenate(shards, axis=0)[:N]
    return np.ascontiguousarray(out)


if __name__ == "__main__":
    _build_nc()
    print("build ok")
